# revision 1
# baseline (speedup 1.0000x reference)
"""MoE SwiGLU experts kernel for Trainium2, 8 NeuronCores.

Strategy: expert parallelism. Host gathers each expert's routed tokens
(top-2-of-8 routing => ~T/4 tokens per expert), pads to a common capacity
C, and each of the 8 cores runs one expert's full SwiGLU FFN on its token
slice. The per-token routing weight and the scatter-add combine happen on
host (they are the unshard step).

On-core math (feature-major / transposed layout, no on-device transposes):
  gateT[I,C] = w1.T @ xT     (lhsT = w1[H,I] tiles, rhs = xT[H,C] tiles)
  upT[I,C]   = w3.T @ xT
  hT         = silu(gateT) * upT          (ACT silu + DVE mul)
  yT[H,C]    = w2.T @ hT     (lhsT = w2[I,H] tiles, rhs = hT tiles)

Matmuls run in float32r (fp32 with 11-bit mantissa, full PE rate at
moving-dim >= 256); inputs are pre-rounded to fp32r on host so hardware
and verifier agree. PSUM accumulation is fp32.

Shapes (hardcoded): T=4096, H=2048, I=4096, E=8, TOP_K=2.
"""

import sys

sys.path.insert(0, "/opt/trn_rl_repo")

import numpy as np

import concourse.bass as bass  # noqa: F401  (registers AP machinery)
import concourse.tile as tile
from concourse import bacc, mybir

F32 = mybir.dt.float32
F32R = mybir.dt.float32r
AFT = mybir.ActivationFunctionType

H = 2048
I = 4096
E = 8
KO = H // 128   # 16 k-tiles over H
MO = I // 128   # 32 m-tiles over I
N_TILE = 512


def _round_fp32r(a: np.ndarray) -> np.ndarray:
    """Round-to-nearest-even to fp32r (11 mantissa bits, low 12 bits zero)."""
    u = np.ascontiguousarray(a, dtype=np.float32).view(np.uint32)
    rb = (u >> 12) & 1
    u = (u + 0x7FF + rb) & 0xFFFFF000
    return u.view(np.float32)


def _build_nc(C: int):
    """Build the per-core Bass program for capacity C (multiple of 512)."""
    n_pass = C // N_TILE
    nc = bacc.Bacc(None, target_bir_lowering=False, debug=False)

    xT_d = nc.dram_tensor("xT", [KO, 128, C], F32R, kind="ExternalInput")
    w1_d = nc.dram_tensor("w1", [MO, 128, KO, 128], F32R, kind="ExternalInput")
    w3_d = nc.dram_tensor("w3", [MO, 128, KO, 128], F32R, kind="ExternalInput")
    w2_d = nc.dram_tensor("w2", [KO, 128, MO, 128], F32R, kind="ExternalInput")
    out_d = nc.dram_tensor("outT", [KO, 128, C], F32, kind="ExternalOutput")

    with tile.TileContext(nc) as tc:
        with (
            tc.tile_pool(name="x", bufs=1) as xpool,
            tc.tile_pool(name="h", bufs=MO) as hpool,
            tc.tile_pool(name="w13", bufs=6) as wpool,
            tc.tile_pool(name="w2", bufs=2) as w2pool,
            tc.tile_pool(name="g", bufs=4) as gpool,
            tc.tile_pool(name="o", bufs=4) as opool,
            tc.tile_pool(name="zb", bufs=1) as zbpool,
            tc.tile_pool(name="ps1", bufs=4, space="PSUM") as ppool,
            tc.tile_pool(name="ps2", bufs=2, space="PSUM") as p2pool,
        ):
            zb = zbpool.tile([128, 1], F32)
            nc.gpsimd.memset(zb[:], 0.0)

            for n in range(n_pass):
                cs = bass.ds(n * N_TILE, N_TILE)
                xt = xpool.tile([128, KO, N_TILE], F32R, tag="x")
                for k in range(KO):
                    nc.sync.dma_start(xt[:, k, :], xT_d[k, :, cs])

                hts = []
                for m in range(MO):
                    w1t = wpool.tile([128, KO * 128], F32R, tag="w")
                    nc.sync.dma_start(w1t[:], w1_d[m])
                    w3t = wpool.tile([128, KO * 128], F32R, tag="w")
                    nc.sync.dma_start(w3t[:], w3_d[m])

                    pg = ppool.tile([128, N_TILE], F32, tag="ps")
                    pu = ppool.tile([128, N_TILE], F32, tag="ps")
                    for k in range(KO):
                        nc.tensor.matmul(
                            pg[:], w1t[:, bass.ts(k, 128)], xt[:, k, :],
                            start=(k == 0), stop=(k == KO - 1),
                        )
                    for k in range(KO):
                        nc.tensor.matmul(
                            pu[:], w3t[:, bass.ts(k, 128)], xt[:, k, :],
                            start=(k == 0), stop=(k == KO - 1),
                        )

                    sg = gpool.tile([128, N_TILE], F32, tag="g")
                    nc.scalar.activation(sg[:], pg[:], AFT.Silu, bias=zb[:])
                    ht = hpool.tile([128, N_TILE], F32R, tag="h")
                    nc.vector.tensor_mul(ht[:], sg[:], pu[:])
                    hts.append(ht)

                for m2 in range(KO):
                    w2t = w2pool.tile([128, MO * 128], F32R, tag="w2")
                    nc.sync.dma_start(w2t[:], w2_d[m2])
                    py = p2pool.tile([128, N_TILE], F32, tag="ps2")
                    for k2 in range(MO):
                        nc.tensor.matmul(
                            py[:], w2t[:, bass.ts(k2, 128)], hts[k2][:],
                            start=(k2 == 0), stop=(k2 == MO - 1),
                        )
                    ot = opool.tile([128, N_TILE], F32, tag="o")
                    nc.vector.tensor_copy(ot[:], py[:])
                    nc.sync.dma_start(out_d[m2, :, cs], ot[:])

    nc.compile()
    return nc


def _prepare(x, expert_weights, w1_stacked, w2_stacked, w3_stacked,
             expert_indices):
    """Host-side routing + per-core input layout prep."""
    T = x.shape[0]
    ids_list, wtok_list = [], []
    for e in range(E):
        msk = expert_indices == e
        tok = np.nonzero(msk.any(axis=1))[0]
        wtok = (expert_weights * msk).sum(axis=1)[tok].astype(np.float32)
        ids_list.append(tok)
        wtok_list.append(wtok)

    cmax = max(len(t) for t in ids_list)
    C = max(N_TILE, -(-cmax // N_TILE) * N_TILE)

    in_maps = []
    for e in range(E):
        tok = ids_list[e]
        xe = np.zeros((C, H), dtype=np.float32)
        xe[: len(tok)] = x[tok]
        xT = _round_fp32r(np.ascontiguousarray(xe.T)).reshape(KO, 128, C)
        w1h = np.ascontiguousarray(
            _round_fp32r(w1_stacked[e]).reshape(KO, 128, MO, 128).transpose(2, 1, 0, 3)
        )
        w3h = np.ascontiguousarray(
            _round_fp32r(w3_stacked[e]).reshape(KO, 128, MO, 128).transpose(2, 1, 0, 3)
        )
        w2h = np.ascontiguousarray(
            _round_fp32r(w2_stacked[e]).reshape(MO, 128, KO, 128).transpose(2, 1, 0, 3)
        )
        in_maps.append({"xT": xT, "w1": w1h, "w3": w3h, "w2": w2h})

    return in_maps, ids_list, wtok_list, C


def _combine(results, ids_list, wtok_list, C, T):
    out = np.zeros((T, H), dtype=np.float32)
    for e in range(E):
        tok = ids_list[e]
        if len(tok) == 0:
            continue
        yT = results[e]["outT"].reshape(H, C)
        out[tok] += wtok_list[e][:, None] * yT[:, : len(tok)].T
    return out


def kernel(**inputs) -> np.ndarray:
    x = np.asarray(inputs["x"], dtype=np.float32)
    expert_weights = np.asarray(inputs["expert_weights"], dtype=np.float32)
    w1_stacked = np.asarray(inputs["w1_stacked"], dtype=np.float32)
    w2_stacked = np.asarray(inputs["w2_stacked"], dtype=np.float32)
    w3_stacked = np.asarray(inputs["w3_stacked"], dtype=np.float32)
    expert_indices = np.asarray(inputs["expert_indices"])

    in_maps, ids_list, wtok_list, C = _prepare(
        x, expert_weights, w1_stacked, w2_stacked, w3_stacked, expert_indices
    )
    nc = _build_nc(C)

    from concourse.bass_utils import run_bass_kernel_spmd

    res = run_bass_kernel_spmd(nc, in_maps, list(range(E)))
    return _combine(res.results, ids_list, wtok_list, C, x.shape[0])


# revision 16
# speedup vs baseline: 1.0283x; 1.0283x over previous
"""MoE SwiGLU experts kernel for Trainium2, 8 NeuronCores.

Strategy: expert parallelism. Host gathers each expert's routed tokens
(top-2-of-8 routing => ~T/4 tokens per expert), pads to a common capacity
C, and each of the 8 cores runs one expert's full SwiGLU FFN on its token
slice. The per-token routing weight and the scatter-add combine happen on
host (they are the unshard step).

On-core math (feature-major / transposed layout, no on-device transposes):
  gateT[I,C] = w1.T @ xT     (lhsT = w1[H,I] tiles, rhs = xT[H,C] tiles)
  upT[I,C]   = w3.T @ xT
  hT         = silu(gateT) * upT          (ACT silu + DVE mul)
  yT[H,C]    = w2.T @ hT     (lhsT = w2[I,H] tiles, rhs = hT tiles)

Matmuls run in float32r (fp32 with 11-bit mantissa, full PE rate at
moving-dim >= 256); inputs are pre-rounded to fp32r on host so hardware
and verifier agree. PSUM accumulation is fp32.

Shapes (hardcoded): T=4096, H=2048, I=4096, E=8, TOP_K=2.
"""

import sys

sys.path.insert(0, "/opt/trn_rl_repo")

import numpy as np

import concourse.bass as bass  # noqa: F401  (registers AP machinery)
import concourse.tile as tile
from concourse import bacc, mybir

F32 = mybir.dt.float32
F32R = mybir.dt.float32r
BF16 = mybir.dt.bfloat16
AFT = mybir.ActivationFunctionType

H = 2048
I = 4096
E = 8
KO = H // 128   # 16 k-tiles over H
MO = I // 128   # 32 m-tiles over I
N_TILE = 512


def _round_fp32r(a: np.ndarray) -> np.ndarray:
    """Round-to-nearest-even to fp32r (11 mantissa bits, low 12 bits zero)."""
    u = np.ascontiguousarray(a, dtype=np.float32).view(np.uint32)
    rb = (u >> 12) & 1
    u = (u + 0x7FF + rb) & 0xFFFFF000
    return u.view(np.float32)


def _build_nc(C: int, body_reps: int = 1, timing_mode: bool = False):
    """Build the per-core Bass program for capacity C (multiple of 512).

    body_reps > 1 repeats the whole computation (same inputs/outputs).
    timing_mode=True declares the big tensors as device-internal DRAM
    (garbage values, nothing shipped through the tunnel) with a tiny
    external output — used only to measure device execution time via the
    marginal cost of extra body_reps.
    """
    n_pass = C // N_TILE
    nc = bacc.Bacc(None, target_bir_lowering=False, debug=False)

    if timing_mode:
        xT_d = nc.dram_tensor("xT", [KO, 128, C], F32R)
        w1_d = nc.dram_tensor("w1", [MO, 128, KO, 128], F32R)
        w3_d = nc.dram_tensor("w3", [MO, 128, KO, 128], F32R)
        w2_d = nc.dram_tensor("w2", [KO, 128, MO, 128], F32R)
        out_d = nc.dram_tensor("outT", [KO, 128, C], F32)
        out_small = nc.dram_tensor("out_small", [1, N_TILE], F32, kind="ExternalOutput")
    else:
        xT_d = nc.dram_tensor("xT", [KO, 128, C], F32R, kind="ExternalInput")
        w1_d = nc.dram_tensor("w1", [MO, 128, KO, 128], F32R, kind="ExternalInput")
        w3_d = nc.dram_tensor("w3", [MO, 128, KO, 128], F32R, kind="ExternalInput")
        w2_d = nc.dram_tensor("w2", [KO, 128, MO, 128], F32R, kind="ExternalInput")
        out_d = nc.dram_tensor("outT", [KO, 128, C], F32, kind="ExternalOutput")

    with tile.TileContext(nc) as tc:
        with (
            tc.tile_pool(name="x", bufs=KO) as xpool,
            tc.tile_pool(name="h", bufs=MO) as hpool,
            tc.tile_pool(name="w13", bufs=6) as wpool,
            tc.tile_pool(name="w2", bufs=2) as w2pool,
            tc.tile_pool(name="g", bufs=4) as gpool,
            tc.tile_pool(name="o", bufs=4) as opool,
            tc.tile_pool(name="zb", bufs=1) as zbpool,
            tc.tile_pool(name="ps1", bufs=4, space="PSUM") as ppool,
            tc.tile_pool(name="ps2", bufs=3, space="PSUM") as p2pool,
        ):
            zb = zbpool.tile([128, 1], F32)
            nc.gpsimd.memset(zb[:], 0.0)

            for n in range(n_pass * body_reps):
                n = n % n_pass
                cs = bass.ds(n * N_TILE, N_TILE)
                def load_w13(m):
                    w1t = wpool.tile([128, KO * 128], F32R, tag="w")
                    nc.sync.dma_start(w1t[:], w1_d[m])
                    w3t = wpool.tile([128, KO * 128], F32R, tag="w")
                    nc.sync.dma_start(w3t[:], w3_d[m])
                    return w1t, w3t

                # m=0 weights first: the first matmul needs w1[0] + xts[0],
                # so don't queue 4MB of xT DMA ahead of them.
                w_first = load_w13(0)
                xts = []
                for k in range(KO):
                    xk = xpool.tile([128, N_TILE], F32R, tag="x")
                    nc.sync.dma_start(xk[:], xT_d[k, :, cs])
                    xts.append(xk)

                hts = []
                for m in range(MO):
                    w1t, w3t = w_first if m == 0 else load_w13(m)

                    pg = ppool.tile([128, N_TILE], F32, tag="ps")
                    pu = ppool.tile([128, N_TILE], F32, tag="ps")
                    for k in range(KO):
                        nc.tensor.matmul(
                            pg[:], w1t[:, bass.ts(k, 128)], xts[k][:],
                            start=(k == 0), stop=(k == KO - 1),
                        )
                    for k in range(KO):
                        nc.tensor.matmul(
                            pu[:], w3t[:, bass.ts(k, 128)], xts[k][:],
                            start=(k == 0), stop=(k == KO - 1),
                        )

                    sg = gpool.tile([128, N_TILE], F32, tag="g")
                    nc.scalar.activation(sg[:], pg[:], AFT.Silu, bias=zb[:])
                    ht = hpool.tile([128, N_TILE], F32R, tag="h")
                    nc.vector.tensor_mul(ht[:], sg[:], pu[:])
                    hts.append(ht)

                for m2 in range(KO):
                    w2t = w2pool.tile([128, MO * 128], F32R, tag="w2")
                    nc.sync.dma_start(w2t[:], w2_d[m2])
                    py = p2pool.tile([128, N_TILE], F32, tag="ps2")
                    for k2 in range(MO):
                        nc.tensor.matmul(
                            py[:], w2t[:, bass.ts(k2, 128)], hts[k2][:],
                            start=(k2 == 0), stop=(k2 == MO - 1),
                        )
                    ot = opool.tile([128, N_TILE], F32, tag="o")
                    nc.vector.tensor_copy(ot[:], py[:])
                    nc.sync.dma_start(out_d[m2, :, cs], ot[:])

            if timing_mode:
                nc.sync.dma_start(out_small[:], ot[:1, :])

    nc.compile()
    return nc


def _prepare(x, expert_weights, w1_stacked, w2_stacked, w3_stacked,
             expert_indices):
    """Host-side routing + per-core input layout prep."""
    T = x.shape[0]
    ids_list, wtok_list = [], []
    for e in range(E):
        msk = expert_indices == e
        tok = np.nonzero(msk.any(axis=1))[0]
        wtok = (expert_weights * msk).sum(axis=1)[tok].astype(np.float32)
        ids_list.append(tok)
        wtok_list.append(wtok)

    cmax = max(len(t) for t in ids_list)
    C = max(N_TILE, -(-cmax // N_TILE) * N_TILE)

    in_maps = []
    for e in range(E):
        tok = ids_list[e]
        xe = np.zeros((C, H), dtype=np.float32)
        xe[: len(tok)] = x[tok]
        xT = _round_fp32r(np.ascontiguousarray(xe.T)).reshape(KO, 128, C)
        w1h = np.ascontiguousarray(
            _round_fp32r(w1_stacked[e]).reshape(KO, 128, MO, 128).transpose(2, 1, 0, 3)
        )
        w3h = np.ascontiguousarray(
            _round_fp32r(w3_stacked[e]).reshape(KO, 128, MO, 128).transpose(2, 1, 0, 3)
        )
        w2h = np.ascontiguousarray(
            _round_fp32r(w2_stacked[e]).reshape(MO, 128, KO, 128).transpose(2, 1, 0, 3)
        )
        in_maps.append({"xT": xT, "w1": w1h, "w3": w3h, "w2": w2h})

    return in_maps, ids_list, wtok_list, C


def _combine(results, ids_list, wtok_list, C, T):
    out = np.zeros((T, H), dtype=np.float32)
    for e in range(E):
        tok = ids_list[e]
        if len(tok) == 0:
            continue
        yT = results[e]["outT"].reshape(H, C)
        out[tok] += wtok_list[e][:, None] * yT[:, : len(tok)].T
    return out


def kernel(**inputs) -> np.ndarray:
    x = np.asarray(inputs["x"], dtype=np.float32)
    expert_weights = np.asarray(inputs["expert_weights"], dtype=np.float32)
    w1_stacked = np.asarray(inputs["w1_stacked"], dtype=np.float32)
    w2_stacked = np.asarray(inputs["w2_stacked"], dtype=np.float32)
    w3_stacked = np.asarray(inputs["w3_stacked"], dtype=np.float32)
    expert_indices = np.asarray(inputs["expert_indices"])

    in_maps, ids_list, wtok_list, C = _prepare(
        x, expert_weights, w1_stacked, w2_stacked, w3_stacked, expert_indices
    )
    nc = _build_nc(C)

    from concourse.bass_utils import run_bass_kernel_spmd

    res = run_bass_kernel_spmd(nc, in_maps, list(range(E)))
    return _combine(res.results, ids_list, wtok_list, C, x.shape[0])
